# revision 10
# baseline (speedup 1.0000x reference)
"""Trainium2 Bass kernel for nn_CONV_tiny_add_partial_558345748883.

Network: 3x [conv5x5(pad2) -> BN -> avgpool2 -> clip01] -> conv4x4(valid) -> BN1d
Input x_in [1024, 3, 32, 32] f32; output [1024, 10] f32.

Strategy (v6)
-------------
- Data parallel: batch 1024 split over 8 NeuronCores (128 samples each).
- Each conv+BN+pool block is algebraically folded into one stride-2 6x6 conv.
- L1/L2 run in fp8e4m3 with DoubleRow perf mode: each matmul carries TWO
  kernel taps (the second K-tile is the same SBUF data at a +1 address
  shift), halving both the column count and the per-column cost.
- L1's BN bias rides in the matmul as a constant-1.0 19th K-row whose
  weight is beta/6, so L1 eviction is a single dual-op clip01
  tensor_scalar (no separate bias/min pass), split across DVE and GpSimd.
- L3/L4 stay fp16 (fp8 there pushes rel-err past the 2e-2 gate).
- Breadth-first wave order L1w0,L1w1,L2w0,L2w1,L3w0,L3w1,L4: each stage's
  PSUM eviction overlaps the other wave's matmuls, removing stage stalls.
- 8 concurrent PE sub-tiles (4 row quads x 2 col tiles) per conv stage.
"""
import os
import sys
import numpy as np
import ml_dtypes

for _p in ("/opt/trn_rl_repo", "/root/.axon_site/_ro/trn_rl_repo"):
    if os.path.isdir(_p) and _p not in sys.path:
        sys.path.append(_p)

import concourse.bass as bass
import concourse.bacc as bacc
import concourse.mybir as mybir
from concourse.tile import TileContext

EPS = 1e-5
N_CORES = 8
F8 = mybir.dt.float8e4
F16 = mybir.dt.float16
F32 = mybir.dt.float32
DT = F16
AF = mybir.ActivationFunctionType
PM = mybir.MatmulPerfMode
NP8 = ml_dtypes.float8_e4m3
NPDT = np.float16

NW = 2    # waves per core
Q = 16    # samples per lane per wave
S = NW * 4 * Q          # samples per core
NQ = NW * Q             # per-lane total samples


def configure(nw, q, use_clears=None):
    global NW, Q, S, NQ, _NC_CACHE
    NW, Q = nw, q
    S = NW * 4 * Q
    NQ = NW * Q
    _NC_CACHE = None


# ----------------------------------------------------------------------------
# Host-side prep
# ----------------------------------------------------------------------------

def _fold_w(w, g, b, m, v):
    """Fold conv5x5 + BN + avgpool2 into stride-2 6x6 weights + bias."""
    inv = g / np.sqrt(v + EPS)
    Wp = np.zeros((w.shape[0], w.shape[1], 6, 6), np.float32)
    for r in (0, 1):
        for s_ in (0, 1):
            Wp[:, :, r:r + 5, s_:s_ + 5] += w
    Wp *= 0.25 * inv[:, None, None, None]
    beta = (b - m * inv).astype(np.float32)
    return Wp.astype(np.float32), beta


def _lane_rep(a, groups=4):
    """Replicate [p, f] into [128, f] across partition groups of 32."""
    out = np.zeros((128, a.shape[1]), np.float32)
    for j in range(groups):
        out[32 * j:32 * j + a.shape[0]] = a
    return out


def host_prep_weights(inputs):
    W1, beta1 = _fold_w(inputs['w1'], inputs['g1'], inputs['b1'], inputs['m1'], inputs['v1'])
    W2, beta2 = _fold_w(inputs['w2'], inputs['g2'], inputs['b2'], inputs['m2'], inputs['v2'])
    W3, beta3 = _fold_w(inputs['w3'], inputs['g3'], inputs['b3'], inputs['m3'], inputs['v3'])
    inv4 = inputs['g4'] / np.sqrt(inputs['v4'] + EPS)
    beta4 = (inputs['b4'] - inputs['m4'] * inv4).astype(np.float32)
    W4 = (inputs['w4'] * inv4[:, None, None, None]).astype(np.float32)

    d = {}
    # L1 lhsT per dx tap f: wl1[dy*3+ci, f*32+co] = W1[co, ci, dy, f]; row 18
    # is the bias row: each of the 6 taps contributes beta1/6.
    wl1 = np.zeros((19, 192), np.float32)
    wl1[0:18] = W1.transpose(2, 1, 3, 0).reshape(18, 6 * 32)
    wl1[18] = np.tile(beta1 / 6.0, 6)
    d['wl1'] = _lane_rep(wl1).astype(NPDT)
    # L2 lhsT per tap t=e*6+f: [32 ci, 32 co]
    d['wl2'] = _lane_rep(W2.transpose(1, 2, 3, 0).reshape(32, 36 * 32)).astype(NPDT)
    # L3 lhsT per tap: [32 ci, 64 co]
    d['wl3'] = _lane_rep(W3.transpose(1, 2, 3, 0).reshape(32, 36 * 64)).astype(np.float16)
    # L4 lhsT per tap t=u*4+v: [64 ci, 10 co], replicated into both row halves
    wl4 = W4.transpose(1, 2, 3, 0).reshape(64, 16 * 10)
    out4 = np.zeros((128, 160), np.float32)
    out4[0:64] = wl4
    out4[64:128] = wl4
    d['wl4'] = out4.astype(np.float16)

    bt = np.zeros((128, 4), np.float32)
    bt[:, 1] = np.tile(beta2, 4)
    bt[:, 2] = np.tile(beta3, 2)
    bt[0:10, 3] = beta4
    d['betas'] = bt
    return d


def host_prep_x(x_core):
    """[S, 3, 32, 32] -> x_l1 [4, 19, NQ, 576] fp8 im2row layout.

    x_l1[lane, dy*3+ci, qg, r*36+c] = xpad[4*qg+lane, ci, 2r+dy, c]
    Row 18 is the constant-1.0 bias row.
    """
    Sc = x_core.shape[0]
    xp = np.zeros((Sc, 3, 36, 36), np.float32)
    xp[:, :, 2:34, 2:34] = x_core
    arr = np.stack([xp[:, :, dy:dy + 32:2, :] for dy in range(6)], axis=1)  # [S,6,3,16,36]
    arr = arr.reshape(Sc, 18, 16 * 36)
    full = np.ones((Sc, 19, 16 * 36), np.float32)
    full[:, 0:18] = arr
    x_l1 = full.reshape(Sc // 4, 4, 19, 576).transpose(1, 2, 0, 3)
    return np.ascontiguousarray(x_l1).astype(NPDT)


# ----------------------------------------------------------------------------
# Bass program
# ----------------------------------------------------------------------------

def _insert_kt(ap, kt_stride, n=2):
    """Insert a [stride, n] k-tile dim right after the partition dim."""
    return bass.AP(ap.tensor, ap.offset, [ap.ap[0], [kt_stride, n]] + list(ap.ap[1:]))


def _pair_w(ap, m):
    """[K, 2*m] weight slice -> [K, 2, m] (adjacent taps)."""
    return bass.AP(ap.tensor, ap.offset, [ap.ap[0], [m, 2], [1, m]])


def build_program():
    nc = bacc.Bacc(target_bir_lowering=False)

    x_l1 = nc.dram_tensor("x_l1", [4, 19, NQ, 576], DT, kind="ExternalInput")
    wl1 = nc.dram_tensor("wl1", [128, 192], DT, kind="ExternalInput")
    wl2 = nc.dram_tensor("wl2", [128, 1152], DT, kind="ExternalInput")
    wl3 = nc.dram_tensor("wl3", [128, 2304], F16, kind="ExternalInput")
    wl4 = nc.dram_tensor("wl4", [128, 160], F16, kind="ExternalInput")
    betas = nc.dram_tensor("betas", [128, 4], F32, kind="ExternalInput")
    y = nc.dram_tensor("y", [10, 4 * NQ], F32, kind="ExternalOutput")

    TAPS = [(e, f) for e in range(6) for f in range(6)]

    with TileContext(nc) as tc:
        with tc.tile_pool(name="consts", bufs=1) as cpool:
            # ---- constants ----
            wl1_t = cpool.tile([128, 192], DT, name="wl1_t")
            wl2_t = cpool.tile([128, 1152], DT, name="wl2_t")
            wl3_t = cpool.tile([128, 2304], F16, name="wl3_t")
            wl4_t = cpool.tile([128, 160], F16, name="wl4_t")
            betas_t = cpool.tile([128, 4], F32, name="betas_t")
            nc.sync.dma_start(wl1_t[:, :], wl1.ap())
            nc.sync.dma_start(betas_t[:, :], betas.ap())
            deferred_w = [1]  # issued after the wave-0 input DMAs

            def flush_weights():
                if not deferred_w:
                    return
                nc.scalar.dma_start(wl2_t[:, :], wl2.ap())
                nc.scalar.dma_start(wl3_t[:, :], wl3.ap())
                nc.scalar.dma_start(wl4_t[:, :], wl4.ap())
                deferred_w.clear()

            # ---- persistent activation tiles ----
            l2in = [cpool.tile([128, Q * 400], DT, name=f"l2in{i}") for i in range(2)]
            l3in = [cpool.tile([128, Q * 144], F16, name=f"l3in{i}") for i in range(2)]
            stagA = cpool.tile([128, NQ * 16], F16, name="stagA")
            stagB = cpool.tile([128, NQ * 16], F16, name="stagB")
            out_sb = cpool.tile([128, 4 * NQ], F32, name="out_sb")

            def memset_borders_l3(t_):
                # l3in sample-major [s(144), yy(12), xx(1)]: 2-wide pad borders
                W_, nrows = 12, 12
                v = t_.rearrange("p (s v) -> p s v", v=W_ * nrows)
                nc.gpsimd.memset(
                    bass.AP(v.tensor, v.offset,
                            [v.ap[0], v.ap[1], [(nrows - 2) * W_, 2], [1, 2 * W_]]),
                    0.0)
                nc.gpsimd.memset(
                    bass.AP(v.tensor, v.offset + 2 * W_,
                            [v.ap[0], v.ap[1], [W_, nrows - 4], [W_ - 2, 2], [1, 2]]),
                    0.0)

            def memset_borders_l2(t_):
                # l2in row-pair-interleaved:
                # addr = c2*3200 + oy*320 + s'*40 + par*20 + xx
                p = t_[:, :].ap[0]
                for c2 in range(2):
                    b = t_.offset + c2 * 3200
                    # rows 0,1 (oy=0) and 18,19 (oy=9): contiguous 320 each
                    nc.gpsimd.memset(
                        bass.AP(t_.tensor, b, [p, [2880, 2], [1, 320]]), 0.0)
                    # side cols: {18,19,20,21} spans right(sp)+left(sp+1)
                    nc.gpsimd.memset(
                        bass.AP(t_.tensor, b + 320 + 18,
                                [p, [320, 8], [20, 15], [1, 4]]), 0.0)
                    # leftover: left cols of sp0, right cols of sp15
                    nc.gpsimd.memset(
                        bass.AP(t_.tensor, b + 320,
                                [p, [320, 8], [318, 2], [1, 2]]), 0.0)

            for t_ in (l2in[0], l2in[1]):
                memset_borders_l2(t_)
            for t_ in (l3in[0], l3in[1]):
                memset_borders_l3(t_)

            with (
                tc.tile_pool(name="l1io", bufs=4) as l1pool,
                tc.tile_pool(name="ps", bufs=8, space="PSUM") as pspool,
            ):
                NSG = Q // 4

                def l1_stage(w):
                    l2t = l2in[w % 2]
                    l1t2 = None
                    for sg in range(NSG):
                        if sg % 2 == 0:
                            nsgl = min(2, NSG - sg) * 4
                            l1t2 = l1pool.tile([128, 8 * 576], DT, name="l1t", tag="l1t")
                            l1d = l1t2.rearrange("p (s v) -> p s v", v=576)
                            q0 = w * Q + 4 * sg
                            for r in range(4):
                                eng = nc.sync if r % 2 == 0 else nc.scalar
                                eng.dma_start(
                                    l1d[32 * r:32 * r + 19, 0:nsgl, :],
                                    x_l1.ap()[r, :, q0:q0 + nsgl, :],
                                )
                            if w == 1 and sg == 2:
                                flush_weights()
                        l1v = l1t2.rearrange("p (s rr cc) -> p s rr cc", s=8, rr=16)
                        so = 4 * (sg % 2)
                        pl1 = [pspool.tile([128, 512], F32, name=f"ps1_{r}", tag="ps")
                               for r in range(4)]
                        for f in range(6):
                            for r in range(4):
                                lhsT = wl1_t[32 * r:32 * r + 19, 32 * f:32 * f + 32]
                                for c in range(2):
                                    a = l1v[32 * r:32 * r + 19,
                                            so + 2 * c:so + 2 * c + 2,
                                            :, f:f + 31:2]
                                    # fold (s,yy): yy spans 16*36=576 == s stride
                                    rhs = bass.AP(a.tensor, a.offset,
                                                  [a.ap[0], [36, 32], [2, 16]])
                                    nc.tensor.matmul(
                                        pl1[r][32 * c:32 * c + 32, :], lhsT, rhs,
                                        start=(f == 0), stop=False,
                                        skip_group_check=True,
                                        tile_position=(32 * r, 32 * c),
                                    )
                        # evac: clip01 into row-pair-interleaved l2in
                        # value (co, c, k, iy, ix) -> addr c2*3200 + oy*320 +
                        #   s'*40 + par*20 + ix, slot s = slot0+k, iy=2oy+par
                        for r in range(4):
                            gb = 64 * (r % 2)
                            slot0 = 4 * sg + 2 * (r // 2)
                            c2, s0p = slot0 // 8, slot0 % 8
                            psl = pl1[r][0:64, :]
                            dsl = l2t[gb:gb + 64, :]
                            d0 = dsl.offset + c2 * 3200 + 320 + s0p * 40 + 2
                            for par in range(2):
                                src = bass.AP(psl.tensor, psl.offset + 16 * par,
                                              [psl.ap[0], [256, 2], [32, 8], [1, 16]])
                                dst = bass.AP(dsl.tensor, d0 + 20 * par,
                                              [dsl.ap[0], [40, 2], [320, 8], [1, 16]])
                                if r < 2:
                                    nc.vector.tensor_scalar(
                                        dst, src, 1.0, 0.0,
                                        mybir.AluOpType.min, mybir.AluOpType.max)
                                else:
                                    nc.scalar.activation(dst, src, AF.Relu,
                                                         bias=0.0, scale=1.0)
                        # min over the ACT-written slots {4sg+2, 4sg+3}
                        mc2, ms0 = (4 * sg + 2) // 8, (4 * sg + 2) % 8
                        mb = l2t.offset + mc2 * 3200 + ms0 * 40
                        mAP = bass.AP(l2t.tensor, mb,
                                      [l2t[:, :].ap[0], [320, 10], [40, 2], [1, 40]])
                        nc.vector.tensor_scalar_min(mAP, mAP, 1.0)

                def l2_stage(w):
                    l2t, l3t = l2in[w % 2], l3in[w % 2]
                    pl2 = [pspool.tile([128, 512], F32, name=f"ps2_{r}", tag="ps")
                           for r in range(4)]
                    H2 = Q // 2
                    # rhs cols (y,s') folded (s' spans 8*40 == y stride 320)
                    for t, (e, f) in enumerate(TAPS):
                        pe, par = e // 2, e % 2
                        for r2 in range(4):
                            lhsT = wl2_t[32 * r2:32 * r2 + 32, 32 * t:32 * t + 32]
                            for c2 in range(2):
                                xsl = l2t[32 * r2:32 * r2 + 32, :]
                                rhs = bass.AP(
                                    xsl.tensor,
                                    xsl.offset + c2 * 3200 + pe * 320 + par * 20 + f,
                                    [xsl.ap[0], [40, 64], [2, 8]])
                                nc.tensor.matmul(
                                    pl2[r2][32 * c2:32 * c2 + 32, 0:H2 * 64], lhsT, rhs,
                                    start=(t == 0), stop=False,
                                    skip_group_check=True,
                                    tile_position=(32 * r2, 32 * c2),
                                )
                    # evac: psum cols are (y, s', x); l3in stays sample-major
                    for r2 in range(4):
                        gb = 64 * (r2 % 2)
                        slot0 = H2 * (r2 // 2)
                        src_ap = pl2[r2][0:64, 0:H2 * 64].rearrange(
                            "p (y k xx) -> p y k xx", y=8, k=H2)
                        dsl = l3t[gb:gb + 64, :]
                        dst = bass.AP(dsl.tensor,
                                      dsl.offset + slot0 * 144 + 2 * 12 + 2,
                                      [dsl.ap[0], [12, 8], [144, H2], [1, 8]])
                        nc.scalar.activation(dst, src_ap, AF.Relu,
                                             bias=betas_t[gb:gb + 64, 1:2], scale=1.0)
                    nc.vector.tensor_scalar_min(l3t[:, :], l3t[:, :], 1.0)

                def l3_stage(w):
                    l3t = l3in[w % 2]
                    l3v = l3t.rearrange("p (s yy xx) -> p s yy xx", s=Q, yy=12)
                    pl3 = [pspool.tile([128, 256], F32, name=f"ps3_{r}", tag="ps")
                           for r in range(4)]
                    for t, (e, f) in enumerate(TAPS):
                        for r3 in range(4):
                            c3 = r3 // 2
                            lhsT = wl3_t[32 * r3:32 * r3 + 32, 64 * t:64 * t + 64]
                            rhs = l3v[32 * r3:32 * r3 + 32, :, e:e + 7:2, f:f + 7:2]
                            nc.tensor.matmul(
                                pl3[r3][64 * c3:64 * c3 + 64, 0:Q * 16], lhsT, rhs,
                                start=(t == 0), stop=(t == 35),
                                skip_group_check=True,
                                tile_position=(32 * r3, 64 * c3),
                            )
                    for r3 in range(4):
                        c3 = r3 // 2
                        stag = stagA if r3 % 2 == 0 else stagB
                        nc.scalar.activation(
                            stag[64 * c3:64 * c3 + 64, w * Q * 16:(w + 1) * Q * 16],
                            pl3[r3][64 * c3:64 * c3 + 64, 0:Q * 16],
                            AF.Relu, bias=betas_t[64 * c3:64 * c3 + 64, 2:3], scale=1.0,
                        )
                    for stag in (stagA, stagB):
                        nc.vector.tensor_scalar_min(
                            stag[:, w * Q * 16:(w + 1) * Q * 16],
                            stag[:, w * Q * 16:(w + 1) * Q * 16], 1.0)

                # breadth-first schedule: evictions overlap the other wave
                for w in range(NW):
                    l1_stage(w)
                for w in range(NW):
                    l2_stage(w)
                for w in range(NW):
                    l3_stage(w)

                # ================= L4 =================
                streams = [(stagA, 0), (stagA, 1), (stagB, 0), (stagB, 1)]
                ps4s = [pspool.tile([128, NQ], F32, name=f"ps4_{k}", tag="ps")
                        for k in range(4)]
                for t in range(16):
                    for k, (stag, r) in enumerate(streams):
                        sv = stag.rearrange("p (n t) -> p n t", t=16)
                        lhsT = wl4_t[64 * r:64 * r + 64, 10 * t:10 * t + 10]
                        rhs = sv[64 * r:64 * r + 64, :, t]
                        nc.tensor.matmul(
                            ps4s[k][0:10, :], lhsT, rhs,
                            start=(t == 0), stop=(t == 15),
                            skip_group_check=True,
                            tile_position=(64 * r, 0),
                        )
                for k in range(4):
                    nc.scalar.activation(
                        out_sb[0:10, k * NQ:(k + 1) * NQ], ps4s[k][0:10, :],
                        AF.Identity, bias=betas_t[0:10, 3:4], scale=1.0,
                    )
                nc.sync.dma_start(y.ap(), out_sb[0:10, :])

        return nc


_NC_CACHE = None


def get_program():
    global _NC_CACHE
    if _NC_CACHE is None:
        nc = build_program()
        if not nc.is_finalized():
            nc.finalize()
        _NC_CACHE = nc
    return _NC_CACHE


def make_in_maps(inputs, n_cores=N_CORES):
    wdict = host_prep_weights(inputs)
    in_maps = []
    for c in range(n_cores):
        x_core = np.asarray(inputs['x_in'][c * S:(c + 1) * S], np.float32)
        m = {'x_l1': host_prep_x(x_core)}
        m.update(wdict)
        in_maps.append(m)
    return in_maps


def _phys_sample(k, n):
    """Output stream k, slot n -> per-core sample index (8-tile mapping)."""
    H2 = Q // 2
    w, m = divmod(n, Q)
    r3 = [0, 2, 1, 3][k]
    g3 = r3
    r2 = 2 * (m // H2) + (g3 // 2)
    n1 = H2 * (g3 % 2) + (m % H2)
    g = r2
    r1 = 2 * ((n1 % 4) // 2) + (g // 2)
    q = 4 * (n1 // 4) + 2 * (g % 2) + (n1 % 2)
    return 4 * Q * w + 4 * q + r1


def assemble_output(results, n_cores=N_CORES):
    """results: list of per-core dicts with y [10, 4*NQ] -> [n_cores*S, 10]."""
    out = np.zeros((n_cores * S, 10), np.float32)
    for c in range(n_cores):
        yc = np.asarray(results[c]['y'])  # [10, 4*NQ]
        for k in range(4):
            for n in range(NQ):
                out[c * S + _phys_sample(k, n), :] = yc[:, k * NQ + n]
    return out


def kernel(**inputs) -> np.ndarray:
    from concourse.bass_utils import run_bass_kernel_spmd
    nc = get_program()
    in_maps = make_in_maps(inputs)
    res = run_bass_kernel_spmd(nc, in_maps, list(range(N_CORES)))
    return assemble_output(res.results)
